# revision 14
# baseline (speedup 1.0000x reference)
"""Causal local (block) attention kernel for Trainium2, 8-core SPMD.

Problem: B=1, T=8192, H=16, D=64, WINDOW=256, LOOK_BACK=1, f32.
Math notes (validated numerically against the reference):
  - The reference applies RoPE with a per-*window* angle to both q and k of the
    same window (including the looked-back k block).  A shared orthogonal
    rotation cancels in q.k, and v is never rotated, so RoPE is skipped.
  - Softmax is computed without max-subtraction (logits are ~N(0,1), safe).

Sharding: batch*heads across 8 cores -> 2 adjacent heads per core, fully
independent.  Each core receives its pre-sliced [8192, 128] (t, 2*64) q/k/v
and produces the matching [8192, 128] output slice.

Per-core dataflow (per block j of 256 rows, heads h in {0,1}):
  - PE transposes q/k natural tiles [128t, 128hd] -> [128hd, 128t] (PSUM),
    DVE copies to SBUF: Q^T, K^T layouts with d on partitions.
  - S^T[kslot, q] = K^T_chunk.T @ Q^T on PE; one PSUM tile [128, 4, 256] per
    head = {c0 x (diag_j | prev_j+1), c1 x (diag_j | prev_j+1)}.
  - ACT: P^T = exp(0.125 * S^T) PSUM->SBUF in one [128,1024] instruction.
  - GPSIMD affine_select zeroes the causal triangles in-place.
  - PV: O[q,65] += P^T_chunk.T @ V' on PE (V' has a ones column -> row sums).
  - DVE: recip of row sums + normalize while copying PSUM->SBUF staging.
  - HWDGE DMA in 1 MiB-class chunks (8 blocks) for q/k/v/out.
"""

from contextlib import ExitStack

import numpy as np

import concourse.bass as bass
import concourse.tile as tile
from concourse import bacc, mybir
from concourse.bass_utils import run_bass_kernel_spmd
from concourse.masks import make_identity

T, HEADS, D = 8192, 16, 64
N_CORES = 8
HPC = HEADS // N_CORES  # heads per core = 2
W = 256  # window size
NBLK = T // W  # 32 blocks
HD = HPC * D  # 128 cols per core slice
P = 128
GB = 8  # blocks per DMA group
NG = NBLK // GB  # 4 groups
GR = GB * W  # rows per group = 2048
TC = GR // P  # t-chunks per group = 16
SCALE = float(D) ** -0.5
F32 = mybir.dt.float32
F32R = mybir.dt.float32r
BF16 = mybir.dt.bfloat16


def _r(ap):
    """Bitcast an fp32 AP to float32r (same bits, full-rate PE matmul mode)."""
    return ap.bitcast(F32R)


def _body(ctx: ExitStack, tc: tile.TileContext, q_ap, k_ap, v_ap, out_ap):
    nc = tc.nc

    const = ctx.enter_context(tc.tile_pool(name="const", bufs=1))
    qpool = ctx.enter_context(tc.tile_pool(name="qring", bufs=2))
    kpool = ctx.enter_context(tc.tile_pool(name="kring", bufs=2))
    vpool = ctx.enter_context(tc.tile_pool(name="vring", bufs=2))
    stpool = ctx.enter_context(tc.tile_pool(name="stage", bufs=2))
    ppool = ctx.enter_context(tc.tile_pool(name="pP", bufs=4))
    qkpool = ctx.enter_context(tc.tile_pool(name="qkT", bufs=2))
    rcpool = ctx.enter_context(tc.tile_pool(name="rc", bufs=4))
    s_psum = ctx.enter_context(tc.tile_pool(name="sps", bufs=2, space="PSUM"))
    t_psum = ctx.enter_context(tc.tile_pool(name="tps", bufs=2, space="PSUM"))
    o_psum = ctx.enter_context(tc.tile_pool(name="ops", bufs=2, space="PSUM"))

    identity = const.tile([P, P], F32)
    make_identity(nc, identity)

    qg, kg, vg = {}, {}, {}

    def load_group(g):
        if g in qg or g >= NG:
            return
        rows = slice(g * GR, (g + 1) * GR)
        qt = qpool.tile([P, TC, P], F32)
        nc.sync.dma_start(
            out=qt, in_=q_ap[rows, :].rearrange("(tc p) c -> p tc c", p=P)
        )
        kt = kpool.tile([P, TC, P], F32)
        nc.sync.dma_start(
            out=kt, in_=k_ap[rows, :].rearrange("(tc p) c -> p tc c", p=P)
        )
        # V' in bf16 (cast during SWDGE DMA) with a ones column for row sums.
        vt = vpool.tile([P, TC, HPC, D + 1], BF16)
        v_src = v_ap[rows, :].rearrange("(tc p) (h d) -> p tc h d", p=P, h=HPC)
        for h in range(HPC):
            nc.gpsimd.dma_start(out=vt[:, :, h, 0:D], in_=v_src[:, :, h, :])
        nc.gpsimd.memset(vt[:, :, :, D : D + 1], 1.0)
        qg[g], kg[g], vg[g] = qt, kt, vt

    def qsl(j, u):  # q natural tile of block j, t-chunk u -> [128, 128]
        return qg[j // GB][:, 2 * (j % GB) + u, :]

    def ksl(j, u):
        return kg[j // GB][:, 2 * (j % GB) + u, :]

    def vsl(j, c, h):  # V' (with ones col) block j, kslot-chunk c, head h
        return vg[j // GB][:, 2 * (j % GB) + c, h, :]

    load_group(0)
    load_group(1)

    # Prologue: Q^T of block 0 (used as "Q^T_j" in iteration 0).
    tq0 = t_psum.tile([P, 4, P], F32, tag="tq")
    for u in (0, 1):
        nc.tensor.transpose(tq0[:, u, :], qsl(0, u), identity)
    qkT_prev = qkpool.tile([P, 4, P], F32R, tag="qkT")
    nc.vector.tensor_copy(qkT_prev[:, 0:2, :], tq0[:, 0:2, :])

    p_prev = {}
    stage = None
    for j in range(NBLK):
        g, bl = j // GB, j % GB
        last = j == NBLK - 1
        if bl == 0:
            load_group(g + 1)
            stage = stpool.tile([P, TC, P], F32)

        # Transposes for this iteration: Q^T_{j+1} (slots 0:2), K^T_j (2:4).
        tq = t_psum.tile([P, 4, P], F32, tag="tq")
        for u in (0, 1):
            if not last:
                nc.tensor.transpose(tq[:, u, :], qsl(j + 1, u), identity)
            nc.tensor.transpose(tq[:, 2 + u, :], ksl(j, u), identity)
        qkT = qkpool.tile([P, 4, P], F32R, tag="qkT")
        if not last:
            nc.vector.tensor_copy(qkT, tq)
        else:
            nc.vector.tensor_copy(qkT[:, 2:4, :], tq[:, 2:4, :])

        for h in range(HPC):
            b = h * D
            # S^T tile: slot 2c = diag of window j, 2c+1 = prev for window j+1.
            s = s_psum.tile([P, 4, W], F32)
            for c in (0, 1):
                stat = qkT[b : b + D, 2 + c, :]  # K^T chunk c: [64, 128]
                nc.tensor.matmul(
                    s[:, 2 * c, :], stat, qkT_prev[b : b + D, 0:2, :]
                )
                if not last:
                    nc.tensor.matmul(
                        s[:, 2 * c + 1, :], stat, qkT[b : b + D, 0:2, :]
                    )

            p = ppool.tile([P, 4, W], BF16)
            if not last:
                nc.scalar.activation(
                    p, s, mybir.ActivationFunctionType.Exp, scale=SCALE
                )
            else:
                sv = s.rearrange("p (c x) n -> p c x n", c=2)[:, :, 0, :]
                pv = p.rearrange("p (c x) n -> p c x n", c=2)[:, :, 0, :]
                nc.scalar.activation(
                    pv, sv, mybir.ActivationFunctionType.Exp, scale=SCALE
                )

            # Causal triangles: keep kslot p <= q col, zero elsewhere.
            for region in (p[:, 0, 0:P], p[:, 2, P:W]):
                nc.gpsimd.affine_select(
                    out=region,
                    in_=region,
                    compare_op=mybir.AluOpType.is_ge,
                    fill=0.0,
                    base=0,
                    pattern=[[1, P]],
                    channel_multiplier=-1,
                )

            o = o_psum.tile([P, 2, D + 1], F32)
            for r in (0, 1):
                mms = []
                if j > 0:
                    mms.append((p_prev[h][:, 1, r * P : (r + 1) * P], vsl(j - 1, 0, h)))
                    mms.append((p_prev[h][:, 3, r * P : (r + 1) * P], vsl(j - 1, 1, h)))
                mms.append((p[:, 0, r * P : (r + 1) * P], vsl(j, 0, h)))
                if r == 1:
                    mms.append((p[:, 2, r * P : (r + 1) * P], vsl(j, 1, h)))
                for i, (lhsT, rhs) in enumerate(mms):
                    nc.tensor.matmul(
                        o[:, r, :],
                        lhsT,
                        rhs,
                        start=(i == 0),
                        stop=(i == len(mms) - 1),
                    )

            rc = rcpool.tile([P, 2], F32)
            nc.vector.reciprocal(rc, o[:, :, D])
            rc_full = rc[:, :]
            rc_b = bass.AP(
                tensor=rc_full.tensor,
                offset=rc_full.offset,
                ap=[rc_full.ap[0], rc_full.ap[1], [0, D]],
            )
            nc.vector.tensor_mul(
                out=stage[:, 2 * bl : 2 * bl + 2, b : b + D],
                in0=o[:, :, 0:D],
                in1=rc_b,
            )
            p_prev[h] = p

        qkT_prev = qkT
        if bl == GB - 1:
            rows = slice(g * GR, (g + 1) * GR)
            nc.sync.dma_start(
                out=out_ap[rows, :].rearrange("(tc p) c -> p tc c", p=P),
                in_=stage,
            )


_NC_CACHE = {}


def _get_module():
    if "nc" not in _NC_CACHE:
        nc = bacc.Bacc(
            "TRN2", target_bir_lowering=False, debug=False, enable_asserts=False
        )
        q_ap = nc.dram_tensor("q", [T, HD], F32, kind="ExternalInput").ap()
        k_ap = nc.dram_tensor("k", [T, HD], F32, kind="ExternalInput").ap()
        v_ap = nc.dram_tensor("v", [T, HD], F32, kind="ExternalInput").ap()
        out_ap = nc.dram_tensor("out", [T, HD], F32, kind="ExternalOutput").ap()
        with tile.TileContext(nc) as tc, ExitStack() as ctx:
            _body(ctx, tc, q_ap, k_ap, v_ap, out_ap)
        nc.compile()
        _NC_CACHE["nc"] = nc
    return _NC_CACHE["nc"]


def _shard(x):
    # (1, T, H, D) -> per-core contiguous [T, 2*D] slices
    x = np.ascontiguousarray(np.asarray(x, dtype=np.float32).reshape(T, HEADS, D))
    return [
        np.ascontiguousarray(x[:, 2 * c : 2 * c + 2, :].reshape(T, HD))
        for c in range(N_CORES)
    ]


def _run(in_maps, **kwargs):
    nc = _get_module()
    return run_bass_kernel_spmd(nc, in_maps, core_ids=list(range(N_CORES)), **kwargs)


def kernel(q, k, v, **run_kwargs):
    qs, ks, vs = _shard(q), _shard(k), _shard(v)
    in_maps = [{"q": qs[c], "k": ks[c], "v": vs[c]} for c in range(N_CORES)]
    res = _run(in_maps, **run_kwargs)
    _NC_CACHE["last_results"] = res
    shards = [res.results[c]["out"].reshape(T, HPC, D) for c in range(N_CORES)]
    out = np.concatenate(shards, axis=1).reshape(1, T, HEADS, D)
    return out


if __name__ == "__main__":
    rng = np.random.default_rng(0)
    q = rng.standard_normal((1, T, HEADS, D), dtype=np.float32)
    k = rng.standard_normal((1, T, HEADS, D), dtype=np.float32)
    v = rng.standard_normal((1, T, HEADS, D), dtype=np.float32)
    out = kernel(q, k, v)
    print("kernel ran, out shape", out.shape, "mean", float(np.abs(out).mean()))


# revision 17
# speedup vs baseline: 1.2642x; 1.2642x over previous
"""Causal local (block) attention kernel for Trainium2, 8-core SPMD.

Problem: B=1, T=8192, H=16, D=64, WINDOW=256, LOOK_BACK=1, f32.
Math notes (validated numerically against the reference):
  - The reference applies RoPE with a per-*window* angle to both q and k of the
    same window (including the looked-back k block).  A shared orthogonal
    rotation cancels in q.k, and v is never rotated, so RoPE is skipped.
  - Softmax is computed without max-subtraction (logits are ~N(0,1), safe).

Sharding: batch*heads across 8 cores -> 2 adjacent heads per core, fully
independent.  Each core receives its pre-sliced [8192, 128] (t, 2*64) q/k/v
and produces the matching [8192, 128] output slice.

Per-core dataflow (per block j of 256 rows, heads h in {0,1}):
  - PE transposes q/k natural tiles [128t, 128hd] -> [128hd, 128t] (PSUM),
    DVE copies to SBUF: Q^T, K^T layouts with d on partitions.
  - S^T[kslot, q] = K^T_chunk.T @ Q^T on PE; one PSUM tile [128, 4, 256] per
    head = {c0 x (diag_j | prev_j+1), c1 x (diag_j | prev_j+1)}.
  - ACT: P^T = exp(0.125 * S^T) PSUM->SBUF in one [128,1024] instruction.
  - GPSIMD affine_select zeroes the causal triangles in-place.
  - PV: O[q,65] += P^T_chunk.T @ V' on PE (V' has a ones column -> row sums).
  - DVE: recip of row sums + normalize while copying PSUM->SBUF staging.
  - HWDGE DMA in 1 MiB-class chunks (8 blocks) for q/k/v/out.
"""

from contextlib import ExitStack

import numpy as np

import concourse.bass as bass
import concourse.tile as tile
from concourse import bacc, mybir
from concourse.bass_utils import run_bass_kernel_spmd
from concourse.masks import make_identity

T, HEADS, D = 8192, 16, 64
N_CORES = 8
HPC = HEADS // N_CORES  # heads per core = 2
W = 256  # window size
NBLK = T // W  # 32 blocks
HD = HPC * D  # 128 cols per core slice
P = 128
GB = 8  # blocks per DMA group
NG = NBLK // GB  # 4 groups
GR = GB * W  # rows per group = 2048
TC = GR // P  # t-chunks per group = 16
SCALE = float(D) ** -0.5
F32 = mybir.dt.float32
F32R = mybir.dt.float32r
BF16 = mybir.dt.bfloat16


def _r(ap):
    """Bitcast an fp32 AP to float32r (same bits, full-rate PE matmul mode)."""
    return ap.bitcast(F32R)


def _body(ctx: ExitStack, tc: tile.TileContext, q_ap, k_ap, v_ap, out_ap):
    nc = tc.nc

    const = ctx.enter_context(tc.tile_pool(name="const", bufs=1))
    qpool = ctx.enter_context(tc.tile_pool(name="qring", bufs=2))
    kpool = ctx.enter_context(tc.tile_pool(name="kring", bufs=2))
    vpool = ctx.enter_context(tc.tile_pool(name="vring", bufs=2))
    stpool = ctx.enter_context(tc.tile_pool(name="stage", bufs=2))
    ppool = ctx.enter_context(tc.tile_pool(name="pP", bufs=4))
    qkpool = ctx.enter_context(tc.tile_pool(name="qkT", bufs=2))
    rcpool = ctx.enter_context(tc.tile_pool(name="rc", bufs=2))
    s_psum = ctx.enter_context(tc.tile_pool(name="sps", bufs=3, space="PSUM"))
    t_psum = ctx.enter_context(tc.tile_pool(name="tps", bufs=1, space="PSUM"))
    o_psum = ctx.enter_context(tc.tile_pool(name="ops", bufs=1, space="PSUM"))

    identity = const.tile([P, P], F32)
    make_identity(nc, identity)

    qg, kg, vg = {}, {}, {}

    def load_group(g):
        if g in qg or g >= NG:
            return
        rows = slice(g * GR, (g + 1) * GR)
        qt = qpool.tile([P, TC, P], F32)
        nc.sync.dma_start(
            out=qt, in_=q_ap[rows, :].rearrange("(tc p) c -> p tc c", p=P)
        )
        kt = kpool.tile([P, TC, P], F32)
        nc.sync.dma_start(
            out=kt, in_=k_ap[rows, :].rearrange("(tc p) c -> p tc c", p=P)
        )
        # V' in bf16 (cast during SWDGE DMA) with a ones column for row sums.
        vt = vpool.tile([P, TC, HPC, D + 1], BF16)
        v_src = v_ap[rows, :].rearrange("(tc p) (h d) -> p tc h d", p=P, h=HPC)
        for h in range(HPC):
            nc.gpsimd.dma_start(out=vt[:, :, h, 0:D], in_=v_src[:, :, h, :])
        nc.gpsimd.memset(vt[:, :, :, D : D + 1], 1.0)
        qg[g], kg[g], vg[g] = qt, kt, vt

    def qsl(j, u):  # q natural tile of block j, t-chunk u -> [128, 128]
        return qg[j // GB][:, 2 * (j % GB) + u, :]

    def ksl(j, u):
        return kg[j // GB][:, 2 * (j % GB) + u, :]

    def vsl(j, c, h):  # V' (with ones col) block j, kslot-chunk c, head h
        return vg[j // GB][:, 2 * (j % GB) + c, h, :]

    load_group(0)
    load_group(1)

    # Prologue: Q^T of block 0 (used as "Q^T_j" in iteration 0).
    tq0 = t_psum.tile([P, 4, P], F32, tag="tq")
    for u in (0, 1):
        nc.tensor.transpose(tq0[:, u, :], qsl(0, u), identity)
    qkT_prev = qkpool.tile([P, 4, P], F32R, tag="qkT")
    nc.vector.tensor_copy(qkT_prev[:, 0:2, :], tq0[:, 0:2, :])

    p_prev = {}
    stage = None
    for j in range(NBLK):
        g, bl = j // GB, j % GB
        last = j == NBLK - 1
        if bl == 0:
            load_group(g + 1)
            stage = stpool.tile([P, TC, P], F32)

        # Transposes for this iteration: Q^T_{j+1} (slots 0:2), K^T_j (2:4).
        tq = t_psum.tile([P, 4, P], F32, tag="tq")
        for u in (0, 1):
            if not last:
                nc.tensor.transpose(tq[:, u, :], qsl(j + 1, u), identity)
            nc.tensor.transpose(tq[:, 2 + u, :], ksl(j, u), identity)
        qkT = qkpool.tile([P, 4, P], F32R, tag="qkT")
        if not last:
            nc.vector.tensor_copy(qkT, tq)
        else:
            nc.vector.tensor_copy(qkT[:, 2:4, :], tq[:, 2:4, :])

        # O tile for both heads: slot = 2*r + h, col 64 = softmax denominator.
        o = o_psum.tile([P, 4, D + 1], F32)
        for h in range(HPC):
            b = h * D
            # S^T tile: slot 2c = diag of window j, 2c+1 = prev for window j+1.
            s = s_psum.tile([P, 4, W], F32)
            for c in (0, 1):
                stat = qkT[b : b + D, 2 + c, :]  # K^T chunk c: [64, 128]
                nc.tensor.matmul(
                    s[:, 2 * c, :], stat, qkT_prev[b : b + D, 0:2, :]
                )
                if not last:
                    nc.tensor.matmul(
                        s[:, 2 * c + 1, :], stat, qkT[b : b + D, 0:2, :]
                    )

            p = ppool.tile([P, 4, W], BF16)
            if not last:
                nc.scalar.activation(
                    p, s, mybir.ActivationFunctionType.Exp, scale=SCALE
                )
            else:
                sv = s.rearrange("p (c x) n -> p c x n", c=2)[:, :, 0, :]
                pv = p.rearrange("p (c x) n -> p c x n", c=2)[:, :, 0, :]
                nc.scalar.activation(
                    pv, sv, mybir.ActivationFunctionType.Exp, scale=SCALE
                )

            # Causal triangles: keep kslot p <= q col, zero elsewhere.
            # One instruction covers both triangle regions (slot0 cols 0:128
            # and slot2 cols 128:256) via a 2D iota pattern that restarts the
            # column counter per region.
            ra = p[:, 0, 0:P]
            region = bass.AP(
                tensor=ra.tensor, offset=ra.offset, ap=[ra.ap[0], [640, 2], [1, P]]
            )
            nc.gpsimd.affine_select(
                out=region,
                in_=region,
                compare_op=mybir.AluOpType.is_ge,
                fill=0.0,
                base=0,
                pattern=[[0, 2], [1, P]],
                channel_multiplier=-1,
            )

            for r in (0, 1):
                mms = []
                if j > 0:
                    mms.append((p_prev[h][:, 1, r * P : (r + 1) * P], vsl(j - 1, 0, h)))
                    mms.append((p_prev[h][:, 3, r * P : (r + 1) * P], vsl(j - 1, 1, h)))
                mms.append((p[:, 0, r * P : (r + 1) * P], vsl(j, 0, h)))
                if r == 1:
                    mms.append((p[:, 2, r * P : (r + 1) * P], vsl(j, 1, h)))
                for i, (lhsT, rhs) in enumerate(mms):
                    nc.tensor.matmul(
                        o[:, 2 * r + h, :],
                        lhsT,
                        rhs,
                        start=(i == 0),
                        stop=(i == len(mms) - 1),
                    )
            p_prev[h] = p

        # Normalize both heads at once: out = O * (1/l), l in column 64.
        rc = rcpool.tile([P, 4], F32)
        nc.vector.reciprocal(rc, o[:, :, D])
        rc_full = rc[:, :]
        rc_b = bass.AP(
            tensor=rc_full.tensor,
            offset=rc_full.offset,
            ap=[rc_full.ap[0], rc_full.ap[1], [0, D]],
        )
        st = stage[:, 2 * bl, 0:1]
        st_out = bass.AP(
            tensor=st.tensor, offset=st.offset, ap=[st.ap[0], [D, 4], [1, D]]
        )
        nc.vector.tensor_mul(out=st_out, in0=o[:, :, 0:D], in1=rc_b)

        qkT_prev = qkT
        if bl == GB - 1:
            rows = slice(g * GR, (g + 1) * GR)
            nc.sync.dma_start(
                out=out_ap[rows, :].rearrange("(tc p) c -> p tc c", p=P),
                in_=stage,
            )


_NC_CACHE = {}


def _get_module():
    if "nc" not in _NC_CACHE:
        nc = bacc.Bacc(
            "TRN2", target_bir_lowering=False, debug=False, enable_asserts=False
        )
        q_ap = nc.dram_tensor("q", [T, HD], F32, kind="ExternalInput").ap()
        k_ap = nc.dram_tensor("k", [T, HD], F32, kind="ExternalInput").ap()
        v_ap = nc.dram_tensor("v", [T, HD], F32, kind="ExternalInput").ap()
        out_ap = nc.dram_tensor("out", [T, HD], F32, kind="ExternalOutput").ap()
        with tile.TileContext(nc) as tc, ExitStack() as ctx:
            _body(ctx, tc, q_ap, k_ap, v_ap, out_ap)
        nc.compile()
        _NC_CACHE["nc"] = nc
    return _NC_CACHE["nc"]


def _shard(x):
    # (1, T, H, D) -> per-core contiguous [T, 2*D] slices
    x = np.ascontiguousarray(np.asarray(x, dtype=np.float32).reshape(T, HEADS, D))
    return [
        np.ascontiguousarray(x[:, 2 * c : 2 * c + 2, :].reshape(T, HD))
        for c in range(N_CORES)
    ]


def _run(in_maps, **kwargs):
    nc = _get_module()
    return run_bass_kernel_spmd(nc, in_maps, core_ids=list(range(N_CORES)), **kwargs)


def kernel(q, k, v, **run_kwargs):
    qs, ks, vs = _shard(q), _shard(k), _shard(v)
    in_maps = [{"q": qs[c], "k": ks[c], "v": vs[c]} for c in range(N_CORES)]
    res = _run(in_maps, **run_kwargs)
    _NC_CACHE["last_results"] = res
    shards = [res.results[c]["out"].reshape(T, HPC, D) for c in range(N_CORES)]
    out = np.concatenate(shards, axis=1).reshape(1, T, HEADS, D)
    return out


if __name__ == "__main__":
    rng = np.random.default_rng(0)
    q = rng.standard_normal((1, T, HEADS, D), dtype=np.float32)
    k = rng.standard_normal((1, T, HEADS, D), dtype=np.float32)
    v = rng.standard_normal((1, T, HEADS, D), dtype=np.float32)
    out = kernel(q, k, v)
    print("kernel ran, out shape", out.shape, "mean", float(np.abs(out).mean()))
